# revision 5
# baseline (speedup 1.0000x reference)
"""Trainium2 Bass kernel for nn_Compression.

Computes: out = X + GAMMA * (P @ (P.T @ X)),  P = softmax(X @ W.T + b)

Strategy (8 NeuronCores, data-parallel over N):
  - Each core owns NLOC = N/8 = 4096 rows of X (32 tiles of 128 rows).
  - Phase A per row-tile: cast X tile to bf16 (kept resident for the
    residual add), PE-transpose it, logits via bf16 matmuls (+ b via a
    K=1 matmul), softmax with fused exp+row-sum on ScalarE, accumulate
    P.T @ X into 4 resident PSUM banks.
  - The PtX accumulation is split into two row-groups (tiles 0-15 and
    16-31) sharing the same PSUM banks.  Group A's partial is drained,
    scaled by GAMMA, cast to bf16 and AllReduced *while phase A keeps
    running* on the second group, hiding that collective entirely.
    Only group B's AllReduce (256 KiB bf16) is exposed at the end, and
    filler PE transposes keep the HAM clock-gate warm across it.
  - Phase B: ptxb = arA + arB (already gamma-scaled), corr = P @ ptxb
    in bf16, residual add against the bf16-resident X, one 512 KiB DMA
    per row-tile.

Precision: the correction term is ~1e-5 of the output magnitude, so
bf16 compute everywhere (including the bf16-resident residual X, which
dominates the error at ~1e-3 relative) is far inside the 2e-2 gate.
"""

import sys

import numpy as np

if "/opt/trn_rl_repo" not in sys.path:
    sys.path.insert(0, "/opt/trn_rl_repo")

N, D, C = 32768, 1024, 256
GAMMA = 1e-4
NCORES = 8
NLOC = N // NCORES  # 4096
P = 128
NT = NLOC // P  # 32
DH = 512
SPLIT = 16  # row-tiles per AllReduce group
NFILL = 120  # PE warm-keeper transposes across the exposed AllReduce

_cache = {}


def _build_nc():
    import concourse.tile as tile
    from concourse import bacc
    import concourse.mybir as mybir
    from concourse.masks import make_identity
    from contextlib import ExitStack

    f32 = mybir.dt.float32
    bf16 = mybir.dt.bfloat16
    AF = mybir.ActivationFunctionType

    nc = bacc.Bacc("TRN2", target_bir_lowering=False, debug=False, num_devices=NCORES)
    X = nc.dram_tensor("X", [NLOC, D], f32, kind="ExternalInput").ap()
    Wt = nc.dram_tensor("Wt", [D, C], f32, kind="ExternalInput").ap()
    bvec = nc.dram_tensor("b", [C], f32, kind="ExternalInput").ap()
    out = nc.dram_tensor("out", [NLOC, D], f32, kind="ExternalOutput").ap()

    with tile.TileContext(nc) as tc, ExitStack() as ctx:
        const = ctx.enter_context(tc.tile_pool(name="const", bufs=1))
        xres = ctx.enter_context(tc.tile_pool(name="xres", bufs=1))
        # f32 staging for X loads; the bf16 cast goes straight into Xall
        xstg = ctx.enter_context(tc.tile_pool(name="xstg", bufs=4))
        work = ctx.enter_context(tc.tile_pool(name="work", bufs=2))
        ppool = ctx.enter_context(tc.tile_pool(name="ppool", bufs=4))
        spool = ctx.enter_context(tc.tile_pool(name="spool", bufs=4))
        stgp = ctx.enter_context(tc.tile_pool(name="stgp", bufs=2))
        opool = ctx.enter_context(tc.tile_pool(name="opool", bufs=3))
        cpool = ctx.enter_context(tc.tile_pool(name="cpool", bufs=3))
        dram = ctx.enter_context(tc.tile_pool(name="dram", bufs=1, space="DRAM"))

        Xall = xres.tile([P, NT, D], bf16)  # bf16-resident X (residual + matmul use)
        Pt = const.tile([P, 2, NLOC], bf16)  # P.T resident

        # X tiles 0-2 first so tile 0's cast/transpose isn't behind the
        # W-chunk DMAs in the queue.
        xf_pre = []
        for i in range(3):
            xf = xstg.tile([P, D], f32, name="xf", tag="xf")
            nc.sync.dma_start(xf[:], X[i * P:(i + 1) * P, :])
            xf_pre.append(xf)

        ident = const.tile([P, P], bf16)
        make_identity(nc, ident)

        # W.T in bf16, [d-within-chunk, k-chunk, c]; 4 DMA chunks, cast on
        # ScalarE to keep DVE free for the first X-tile casts.
        Wt_sb = const.tile([P, 8, C], bf16)
        with tc.tile_pool(name="wtmp", bufs=1) as wtmp:
            wt_f = wtmp.tile([P, 8, C], f32)
            wt_r = Wt.rearrange("(k p) c -> p k c", p=P)
            for q in range(4):
                nc.sync.dma_start(wt_f[:, 2 * q:2 * q + 2, :], wt_r[:, 2 * q:2 * q + 2, :])
                nc.scalar.copy(Wt_sb[:, 2 * q:2 * q + 2, :], wt_f[:, 2 * q:2 * q + 2, :])

        ones1 = const.tile([1, P], bf16)
        nc.vector.memset(ones1[:], 1.0)
        b_sb = const.tile([1, C], bf16)
        with tc.tile_pool(name="btmp", bufs=1) as btmp:
            b_f = btmp.tile([1, C], f32)
            nc.sync.dma_start(b_f[:], bvec.rearrange("(o c) -> o c", o=1))
            nc.vector.tensor_copy(b_sb[:], b_f[:])

        # Two AllReduces (bf16, gamma pre-folded): group A hidden under
        # phase A's second half, group B exposed at the end.
        ar_in = [dram.tile([C, D], bf16, name=f"ar_in{g}") for g in range(2)]
        ar_out = [
            dram.tile([C, D], bf16, addr_space="Shared", name=f"ar_out{g}")
            for g in range(2)
        ]

        # ---- phase A: software-pipelined over row-tiles ----
        def s_load(i):
            if i < 3:
                xf = xf_pre[i]
            else:
                xf = xstg.tile([P, D], f32, name="xf", tag="xf")
                nc.sync.dma_start(xf[:], X[i * P:(i + 1) * P, :])
            xb = Xall[:, i, :]
            nc.vector.tensor_copy(xb, xf[:])
            return xb

        def s_transpose(i, xb):
            xt = work.tile([P, D], bf16, name="xt", tag="xt")
            trp = psA.tile([P, D], bf16, name="trp", tag="trp")
            for k in range(8):
                nc.tensor.matmul(
                    trp[:, k * P:(k + 1) * P],
                    xb[:, k * P:(k + 1) * P],
                    ident[:],
                    is_transpose=True,
                    start=(k == 0),
                    stop=(k == 7),
                )
            nc.scalar.copy(xt[:], trp[:])
            return xt

        def s_logits(i, xt):
            lg = psL.tile([P, C], f32, name="lg", tag="lg")
            for k in range(8):
                nc.tensor.matmul(
                    lg[:],
                    xt[:, k * P:(k + 1) * P],
                    Wt_sb[:, k, :],
                    start=(k == 0),
                    stop=False,
                )
            nc.tensor.matmul(lg[:], ones1[:], b_sb[:], start=False, stop=True)
            return lg

        def s_softmax(i, lg):
            # |logits| <= ~10 so exp is safe without max-subtraction
            p_sb = ppool.tile([P, C], f32, name="p_sb", tag="p")
            ssum = spool.tile([P, 1], f32, name="ssum", tag="s")
            nc.scalar.activation(p_sb[:], lg[:], AF.Exp, accum_out=ssum[:])
            rinv = spool.tile([P, 1], f32, name="rinv", tag="r")
            nc.vector.reciprocal(rinv[:], ssum[:])
            p_bf = ppool.tile([P, C], bf16, name="p_bf", tag="pb")
            nc.vector.tensor_scalar_mul(p_bf[:], p_sb[:], rinv[:])
            return p_bf

        def s_ptx(i, p_bf, xb):
            first = i in (0, SPLIT)
            last = i in (SPLIT - 1, NT - 1)
            for c in range(2):
                for h in range(2):
                    nc.tensor.matmul(
                        ptx_ps[2 * c + h][:],
                        p_bf[:, c * P:(c + 1) * P],
                        xb[:, h * DH:(h + 1) * DH],
                        start=first,
                        stop=last,
                    )
            ptp = psA.tile([P, C], bf16, name="ptp", tag="trp")
            for c in range(2):
                nc.tensor.matmul(
                    ptp[:, c * P:(c + 1) * P],
                    p_bf[:, c * P:(c + 1) * P],
                    ident[:],
                    is_transpose=True,
                    start=(c == 0),
                    stop=(c == 1),
                )
            nc.scalar.copy(
                Pt[:, :, i * P:(i + 1) * P],
                ptp[:].rearrange("p (c n) -> p c n", c=2),
            )

        def drain_and_reduce(g):
            # PSUM -> SBUF with the GAMMA fold + bf16 cast (split across
            # ACT and DVE), then stage to DRAM and AllReduce.
            stg = stgp.tile([P, 2, D], bf16, name=f"stg{g}", tag="stg")
            for c in range(2):
                for h in range(2):
                    dst = stg[:, c, h * DH:(h + 1) * DH]
                    if h == 0:
                        nc.scalar.mul(dst, ptx_ps[2 * c][:], GAMMA)
                    else:
                        nc.vector.tensor_scalar_mul(dst, ptx_ps[2 * c + 1][:], GAMMA)
            nc.sync.dma_start(ar_in[g].rearrange("(c p) d -> p c d", p=P), stg[:])
            nc.gpsimd.collective_compute(
                "AllReduce",
                mybir.AluOpType.add,
                replica_groups=[list(range(NCORES))],
                ins=[ar_in[g][:].opt()],
                outs=[ar_out[g][:].opt()],
            )

        with tc.tile_pool(name="psA", bufs=3, space="PSUM") as psA, \
             tc.tile_pool(name="psL", bufs=1, space="PSUM") as psL, \
             tc.tile_pool(name="psX", bufs=1, space="PSUM") as psX:
            ptx_ps = [
                psX.tile([P, DH], f32, name=f"ptx_{c}_{h}", tag=f"ptx_{c}_{h}")
                for c in range(2)
                for h in range(2)
            ]
            # 2-step skew between softmax(i) and ptx(i): the ~1.1us ScalarE
            # exp latency hides under transposes + the previous ptx + the
            # next logits block instead of stalling the PE.
            xb0 = s_load(0)
            xt0 = s_transpose(0, xb0)
            xb1 = s_load(1)
            state = {0: (xb0, xt0, None), 1: (xb1, None, None)}
            for i in range(NT):
                xb_i, xt_i, _ = state[i]
                lg = s_logits(i, xt_i)
                p_bf = s_softmax(i, lg)
                state[i] = (xb_i, xt_i, p_bf)
                if i + 1 < NT:
                    xb_n, _, _ = state[i + 1]
                    state[i + 1] = (xb_n, s_transpose(i + 1, xb_n), None)
                if i + 2 < NT:
                    state[i + 2] = (s_load(i + 2), None, None)
                if i >= 2:
                    xb_p, _, p_bf_p = state.pop(i - 2)
                    s_ptx(i - 2, p_bf_p, xb_p)
                    if i - 2 == SPLIT - 1:
                        drain_and_reduce(0)
            for i in (NT - 2, NT - 1):
                xb_l, _, p_bf_l = state.pop(i)
                s_ptx(i, p_bf_l, xb_l)
            drain_and_reduce(1)

        # ---- exposed-collective window + phase B ----
        # Keep the HAM clock-gate warm while AllReduce B flies.
        with tc.tile_pool(name="psF", bufs=1, space="PSUM") as psF:
            ftile = psF.tile([P, P], bf16, name="fill", tag="fill")
            for _ in range(NFILL):
                nc.tensor.matmul(
                    ftile[:], ident[:], ident[:], is_transpose=True,
                    start=True, stop=True,
                )

        with tc.tile_pool(name="psB", bufs=4, space="PSUM") as psB:
            pa = const.tile([P, 2, D], bf16, name="pa")
            pb = const.tile([P, 2, D], bf16, name="pb")
            nc.sync.dma_start(pa[:], ar_out[0].rearrange("(c p) d -> p c d", p=P))
            nc.sync.dma_start(pb[:], ar_out[1].rearrange("(c p) d -> p c d", p=P))
            # combine per D-half on separate engines so h0 unblocks early
            ptxb = [const.tile([P, 2, DH], bf16, name=f"ptxb{h}") for h in range(2)]
            nc.vector.tensor_add(ptxb[0][:], pa[:, :, 0:DH], pb[:, :, 0:DH])
            nc.gpsimd.tensor_add(ptxb[1][:], pa[:, :, DH:D], pb[:, :, DH:D])

            for i in range(NT):
                cor = psB.tile([P, 2, DH], f32, name="cor", tag="cor")
                for h in range(2):
                    for c in range(2):
                        nc.tensor.matmul(
                            cor[:, h, :], Pt[:, c, i * P:(i + 1) * P],
                            ptxb[h][:, c, :],
                            start=(c == 0), stop=(c == 1),
                        )
                o_sb = opool.tile([P, 2, DH], f32, name="o_sb", tag="o")
                # h0: DVE adds straight from PSUM; h1: ACT drains PSUM then
                # GpSimd adds in SBUF, spreading phase B over three engines.
                nc.vector.tensor_add(o_sb[:, 0, :], cor[:, 0, :], Xall[:, i, 0:DH])
                c1s = cpool.tile([P, DH], f32, name="c1s", tag="c1s")
                nc.scalar.copy(c1s[:], cor[:, 1, :])
                nc.gpsimd.tensor_add(o_sb[:, 1, :], c1s[:], Xall[:, i, DH:D])
                nc.sync.dma_start(
                    out[i * P:(i + 1) * P, :].rearrange("p (h d) -> p h d", h=2),
                    o_sb[:],
                )

    nc.finalize()
    return nc


def _run(inputs, trace=False, **kwargs):
    from concourse import bass_utils

    if "nc" not in _cache:
        _cache["nc"] = _build_nc()
    nc = _cache["nc"]

    X = np.ascontiguousarray(np.asarray(inputs["X"], dtype=np.float32))
    W = np.ascontiguousarray(np.asarray(inputs["W"], dtype=np.float32))
    b = np.ascontiguousarray(np.asarray(inputs["b"], dtype=np.float32))
    Wt = np.ascontiguousarray(W.T)

    in_maps = [
        {"X": X[i * NLOC:(i + 1) * NLOC], "Wt": Wt, "b": b} for i in range(NCORES)
    ]
    res = bass_utils.run_bass_kernel_spmd(
        nc, in_maps, core_ids=list(range(NCORES)), trace=trace, **kwargs
    )
    outp = np.concatenate([res.results[i]["out"] for i in range(NCORES)], axis=0)
    return outp, res


def kernel(**inputs):
    outp, _ = _run(inputs, trace=False)
    return outp


# revision 7
# speedup vs baseline: 1.1143x; 1.1143x over previous
"""Trainium2 Bass kernel for nn_Compression.

Computes: out = X + GAMMA * (P @ (P.T @ X)),  P = softmax(X @ W.T + b)

Strategy (8 NeuronCores, data-parallel over N):
  - Each core owns NLOC = N/8 = 4096 rows of X (32 tiles of 128 rows).
  - Phase A per row-tile: cast X tile to bf16 (kept resident for the
    residual add), PE-transpose it, logits via bf16 matmuls (+ b via a
    K=1 matmul), softmax with fused exp+row-sum on ScalarE, accumulate
    P.T @ X into 4 resident PSUM banks.
  - The PtX accumulation is split into two row-groups (tiles 0-15 and
    16-31) sharing the same PSUM banks.  Group A's partial is drained,
    cast to fp8e4 and AllReduced *while phase A keeps running* on the
    second group, hiding that collective entirely.  Only group B's
    AllReduce (256 KiB fp8) is exposed at the end; filler PE
    transposes reading the group-B stage (so the scheduler cannot
    hoist them into phase A) keep the HAM clock-gate warm across it.
  - GAMMA is folded into the resident P.T copy, so phase B is just
    ptxb = arA + arB, corr = (gamma*P) @ ptxb, residual add against
    the bf16-resident X, one 512 KiB DMA per row-tile.

Precision: the correction term is ~1e-5 of the output magnitude, so
fp8/bf16 compute of it is harmless; the bf16-resident residual X
dominates the error at ~1.7e-3 relative vs the 2e-2 gate.
"""

import sys

import numpy as np

if "/opt/trn_rl_repo" not in sys.path:
    sys.path.insert(0, "/opt/trn_rl_repo")

N, D, C = 32768, 1024, 256
GAMMA = 1e-4
NCORES = 8
NLOC = N // NCORES  # 4096
P = 128
NT = NLOC // P  # 32
DH = 512
SPLIT = 16  # row-tiles per AllReduce group
NFILL = 180  # PE warm-keeper transposes across the exposed AllReduce

_cache = {}


def _build_nc():
    import concourse.tile as tile
    from concourse import bacc
    import concourse.mybir as mybir
    from concourse.masks import make_identity
    from contextlib import ExitStack

    f32 = mybir.dt.float32
    bf16 = mybir.dt.bfloat16
    f8 = mybir.dt.float8e4
    AF = mybir.ActivationFunctionType

    nc = bacc.Bacc("TRN2", target_bir_lowering=False, debug=False, num_devices=NCORES)
    X = nc.dram_tensor("X", [NLOC, D], f32, kind="ExternalInput").ap()
    Wt = nc.dram_tensor("Wt", [D, C], f32, kind="ExternalInput").ap()
    bvec = nc.dram_tensor("b", [C], f32, kind="ExternalInput").ap()
    out = nc.dram_tensor("out", [NLOC, D], f32, kind="ExternalOutput").ap()

    with tile.TileContext(nc) as tc, ExitStack() as ctx:
        const = ctx.enter_context(tc.tile_pool(name="const", bufs=1))
        xres = ctx.enter_context(tc.tile_pool(name="xres", bufs=1))
        # f32 staging for X loads; the bf16 cast goes straight into Xall
        xstg = ctx.enter_context(tc.tile_pool(name="xstg", bufs=4))
        work = ctx.enter_context(tc.tile_pool(name="work", bufs=2))
        ppool = ctx.enter_context(tc.tile_pool(name="ppool", bufs=4))
        spool = ctx.enter_context(tc.tile_pool(name="spool", bufs=4))
        stgp = ctx.enter_context(tc.tile_pool(name="stgp", bufs=2))
        opool = ctx.enter_context(tc.tile_pool(name="opool", bufs=3))
        cpool = ctx.enter_context(tc.tile_pool(name="cpool", bufs=3))
        dram = ctx.enter_context(tc.tile_pool(name="dram", bufs=1, space="DRAM"))

        Xall = xres.tile([P, NT, D], bf16)  # bf16-resident X (residual + matmul use)
        Pt = const.tile([P, 2, NLOC], bf16)  # gamma * P.T, resident

        # X tiles 0-2 first so tile 0's cast/transpose isn't behind the
        # W-chunk DMAs in the queue.
        xf_pre = []
        for i in range(3):
            xf = xstg.tile([P, D], f32, name="xf", tag="xf")
            nc.sync.dma_start(xf[:], X[i * P:(i + 1) * P, :])
            xf_pre.append(xf)

        ident = const.tile([P, P], bf16)
        make_identity(nc, ident)

        # W.T in bf16, [d-within-chunk, k-chunk, c]; 4 DMA chunks, cast on
        # ScalarE to keep DVE free for the first X-tile casts.
        Wt_sb = const.tile([P, 8, C], bf16)
        with tc.tile_pool(name="wtmp", bufs=1) as wtmp:
            wt_f = wtmp.tile([P, 8, C], f32)
            wt_r = Wt.rearrange("(k p) c -> p k c", p=P)
            for q in range(4):
                nc.sync.dma_start(wt_f[:, 2 * q:2 * q + 2, :], wt_r[:, 2 * q:2 * q + 2, :])
                nc.scalar.copy(Wt_sb[:, 2 * q:2 * q + 2, :], wt_f[:, 2 * q:2 * q + 2, :])

        ones1 = const.tile([1, P], bf16)
        nc.vector.memset(ones1[:], 1.0)
        b_sb = const.tile([1, C], bf16)
        with tc.tile_pool(name="btmp", bufs=1) as btmp:
            b_f = btmp.tile([1, C], f32)
            nc.sync.dma_start(b_f[:], bvec.rearrange("(o c) -> o c", o=1))
            nc.vector.tensor_copy(b_sb[:], b_f[:])

        # Two AllReduces (fp8e4, unscaled partials): group A hidden under
        # phase A's second half, group B exposed at the end.
        ar_in = [dram.tile([C, D], f8, name=f"ar_in{g}") for g in range(2)]
        ar_out = [
            dram.tile([C, D], f8, addr_space="Shared", name=f"ar_out{g}")
            for g in range(2)
        ]

        # ---- phase A: software-pipelined over row-tiles ----
        def s_load(i):
            if i < 3:
                xf = xf_pre[i]
            else:
                xf = xstg.tile([P, D], f32, name="xf", tag="xf")
                nc.sync.dma_start(xf[:], X[i * P:(i + 1) * P, :])
            xb = Xall[:, i, :]
            nc.vector.tensor_copy(xb, xf[:])
            return xb

        def s_transpose(i, xb):
            xt = work.tile([P, D], bf16, name="xt", tag="xt")
            trp = psA.tile([P, D], bf16, name="trp", tag="trp")
            for k in range(8):
                nc.tensor.matmul(
                    trp[:, k * P:(k + 1) * P],
                    xb[:, k * P:(k + 1) * P],
                    ident[:],
                    is_transpose=True,
                    start=(k == 0),
                    stop=(k == 7),
                )
            nc.scalar.copy(xt[:], trp[:])
            return xt

        def s_logits(i, xt):
            lg = psL.tile([P, C], f32, name="lg", tag="lg")
            for k in range(8):
                nc.tensor.matmul(
                    lg[:],
                    xt[:, k * P:(k + 1) * P],
                    Wt_sb[:, k, :],
                    start=(k == 0),
                    stop=False,
                )
            nc.tensor.matmul(lg[:], ones1[:], b_sb[:], start=False, stop=True)
            return lg

        def s_softmax(i, lg):
            # |logits| <= ~10 so exp is safe without max-subtraction
            p_sb = ppool.tile([P, C], f32, name="p_sb", tag="p")
            ssum = spool.tile([P, 1], f32, name="ssum", tag="s")
            nc.scalar.activation(p_sb[:], lg[:], AF.Exp, accum_out=ssum[:])
            rinv = spool.tile([P, 1], f32, name="rinv", tag="r")
            nc.vector.reciprocal(rinv[:], ssum[:])
            p_bf = ppool.tile([P, C], bf16, name="p_bf", tag="pb")
            nc.vector.tensor_scalar_mul(p_bf[:], p_sb[:], rinv[:])
            return p_bf

        def s_ptx(i, p_bf, xb):
            first = i in (0, SPLIT)
            last = i in (SPLIT - 1, NT - 1)
            for c in range(2):
                for h in range(2):
                    nc.tensor.matmul(
                        ptx_ps[2 * c + h][:],
                        p_bf[:, c * P:(c + 1) * P],
                        xb[:, h * DH:(h + 1) * DH],
                        start=first,
                        stop=last,
                    )
            ptp = psA.tile([P, C], bf16, name="ptp", tag="trp")
            for c in range(2):
                nc.tensor.matmul(
                    ptp[:, c * P:(c + 1) * P],
                    p_bf[:, c * P:(c + 1) * P],
                    ident[:],
                    is_transpose=True,
                    start=(c == 0),
                    stop=(c == 1),
                )
            # gamma folded here: the resident P.T is pre-scaled so phase B
            # needs no separate scale pass.
            nc.scalar.mul(
                Pt[:, :, i * P:(i + 1) * P],
                ptp[:].rearrange("p (c n) -> p c n", c=2),
                GAMMA,
            )

        def drain_and_reduce(g):
            # PSUM -> SBUF fp8 casts (split across ACT and DVE), each
            # quadrant DMA'd to DRAM as soon as its cast lands, then the
            # AllReduce trigger.
            stg = stgp.tile([P, 2, D], f8, name=f"stg{g}", tag="stg")
            ar_v = ar_in[g].rearrange("(c p) d -> p c d", p=P)
            for c in range(2):
                for h in range(2):
                    dst = stg[:, c, h * DH:(h + 1) * DH]
                    src = ptx_ps[2 * c + h]
                    if h == 0:
                        nc.scalar.copy(dst, src[:])
                    else:
                        nc.vector.tensor_copy(dst, src[:])
                    nc.sync.dma_start(ar_v[:, c, h * DH:(h + 1) * DH], dst)
            nc.gpsimd.collective_compute(
                "AllReduce",
                mybir.AluOpType.add,
                replica_groups=[list(range(NCORES))],
                ins=[ar_in[g][:].opt()],
                outs=[ar_out[g][:].opt()],
            )
            return stg

        with tc.tile_pool(name="psA", bufs=3, space="PSUM") as psA, \
             tc.tile_pool(name="psL", bufs=1, space="PSUM") as psL, \
             tc.tile_pool(name="psX", bufs=1, space="PSUM") as psX:
            ptx_ps = [
                psX.tile([P, DH], f32, name=f"ptx_{c}_{h}", tag=f"ptx_{c}_{h}")
                for c in range(2)
                for h in range(2)
            ]
            # 2-step skew between softmax(i) and ptx(i): the ~1.1us ScalarE
            # exp latency hides under transposes + the previous ptx + the
            # next logits block instead of stalling the PE.
            xb0 = s_load(0)
            xt0 = s_transpose(0, xb0)
            xb1 = s_load(1)
            state = {0: (xb0, xt0, None), 1: (xb1, None, None)}
            for i in range(NT):
                xb_i, xt_i, _ = state[i]
                lg = s_logits(i, xt_i)
                p_bf = s_softmax(i, lg)
                state[i] = (xb_i, xt_i, p_bf)
                if i >= 2:
                    xb_p, _, p_bf_p = state.pop(i - 2)
                    s_ptx(i - 2, p_bf_p, xb_p)
                    if i - 2 == SPLIT - 1:
                        drain_and_reduce(0)
                if i + 1 < NT:
                    xb_n, _, _ = state[i + 1]
                    state[i + 1] = (xb_n, s_transpose(i + 1, xb_n), None)
                if i + 2 < NT:
                    state[i + 2] = (s_load(i + 2), None, None)
            for i in (NT - 2, NT - 1):
                xb_l, _, p_bf_l = state.pop(i)
                s_ptx(i, p_bf_l, xb_l)
            stg1 = drain_and_reduce(1)

        # ---- exposed-collective window + phase B ----
        # Keep the HAM clock-gate warm while AllReduce B flies.  The
        # fillers read the group-B stage so the scheduler cannot hoist
        # them before the end of phase A.
        with tc.tile_pool(name="psF", bufs=1, space="PSUM") as psF:
            ftile = psF.tile([P, P], bf16, name="fill", tag="fill")
            stgb = stg1[:].bitcast(bf16)  # [P, 2, DH] view of the fp8 stage
            for f in range(NFILL):
                src = stgb[:, f % 2, (f % 4) * P:(f % 4 + 1) * P]
                nc.tensor.matmul(
                    ftile[:], src, ident[:], is_transpose=True,
                    start=True, stop=True,
                )

        with tc.tile_pool(name="psB", bufs=4, space="PSUM") as psB:
            pa = const.tile([P, 2, D], f8, name="pa")
            pb = const.tile([P, 2, D], f8, name="pb")
            nc.sync.dma_start(pa[:], ar_out[0].rearrange("(c p) d -> p c d", p=P))
            nc.sync.dma_start(pb[:], ar_out[1].rearrange("(c p) d -> p c d", p=P))
            # combine per D-half on separate engines so h0 unblocks early
            ptxb = [const.tile([P, 2, DH], bf16, name=f"ptxb{h}") for h in range(2)]
            nc.vector.tensor_add(ptxb[0][:], pa[:, :, 0:DH], pb[:, :, 0:DH])
            nc.gpsimd.tensor_add(ptxb[1][:], pa[:, :, DH:D], pb[:, :, DH:D])

            for i in range(NT):
                cor = psB.tile([P, 2, DH], f32, name="cor", tag="cor")
                for h in range(2):
                    for c in range(2):
                        nc.tensor.matmul(
                            cor[:, h, :], Pt[:, c, i * P:(i + 1) * P],
                            ptxb[h][:, c, :],
                            start=(c == 0), stop=(c == 1),
                        )
                # single ACT drain frees the PSUM banks quickly; the adds
                # then run in SBUF on DVE / GpSimd.
                cs = cpool.tile([P, 2, DH], f32, name="cs", tag="cs")
                nc.scalar.copy(cs[:], cor[:])
                o_sb = opool.tile([P, 2, DH], f32, name="o_sb", tag="o")
                nc.vector.tensor_add(o_sb[:, 0, :], cs[:, 0, :], Xall[:, i, 0:DH])
                nc.gpsimd.tensor_add(o_sb[:, 1, :], cs[:, 1, :], Xall[:, i, DH:D])
                nc.sync.dma_start(
                    out[i * P:(i + 1) * P, :].rearrange("p (h d) -> p h d", h=2),
                    o_sb[:],
                )

    nc.finalize()
    return nc


def _run(inputs, trace=False, **kwargs):
    from concourse import bass_utils

    if "nc" not in _cache:
        _cache["nc"] = _build_nc()
    nc = _cache["nc"]

    X = np.ascontiguousarray(np.asarray(inputs["X"], dtype=np.float32))
    W = np.ascontiguousarray(np.asarray(inputs["W"], dtype=np.float32))
    b = np.ascontiguousarray(np.asarray(inputs["b"], dtype=np.float32))
    Wt = np.ascontiguousarray(W.T)

    in_maps = [
        {"X": X[i * NLOC:(i + 1) * NLOC], "Wt": Wt, "b": b} for i in range(NCORES)
    ]
    res = bass_utils.run_bass_kernel_spmd(
        nc, in_maps, core_ids=list(range(NCORES)), trace=trace, **kwargs
    )
    outp = np.concatenate([res.results[i]["out"] for i in range(NCORES)], axis=0)
    return outp, res


def kernel(**inputs):
    outp, _ = _run(inputs, trace=False)
    return outp
